# revision 4
# baseline (speedup 1.0000x reference)
"""Bahdanau attention Trainium2 kernel.

Shapes (full): hidden (32,1024) f32, encoder_outputs (32,2048,1024) f32,
mask (32,2048) i32, W_h/W_e (1024,1024) f32, b_h/b_e/v (1024,) f32.
Outputs: context (32,1024) f32, attention_weights (32,2048) f32.

Sharding: data-parallel over batch B across 8 cores (4 batches/core);
projection weights replicated.

Per-core pipeline (all big compute in bf16 on the PE):
  h_projT = W_h^T @ hidden^T + b_h + b_e          (tiny, startup)
  per s-block of 1024 (2 per batch, 8 per core):
    natbf  <- gpsimd cast-DMA f32->bf16 of enc rows      [128,1024] x8
    bt     <- xbar transpose (3D out) of natbf           [128,8(et),1024(s)]
    e_projT[h,s] = sum_et W_e[et]^T @ bt[et]  (PSUM f32)
    energy = tanh(e_projT + bias)  (ACT, per-partition bias)
    logits[s] = sum_ht v[ht]^T @ energy[ht]   (M=1 matmuls)
    p = exp(logits - |v|_1) * mask   (ACT exp + DVE mask-mul; no row max needed
                                      since |logit| <= |v|_1)
    pT <- tiny DRAM round-trip xbar ( [16,128] -> [128,16] )
    ctx += sum_j pT[:,j]^T @ natbf[j]          (M=1 matmuls, deferred 1 block)
  ctx_out = ctx * (1/sum p);  w_out = p * (1/sum p)
"""

import sys

for _p in ("/opt/trn_rl_repo", "/root/.axon_site/_ro/trn_rl_repo"):
    if _p not in sys.path:
        sys.path.insert(0, _p)

import numpy as np

import concourse.bass as bass
import concourse.bacc as bacc
import concourse.mybir as mybir
import concourse.tile as tile

F32 = mybir.dt.float32
BF16 = mybir.dt.bfloat16
I32 = mybir.dt.int32
AF = mybir.ActivationFunctionType

N_CORES = 8
B_FULL, S, D = 32, 2048, 1024
B_LOC = B_FULL // N_CORES          # 4 batches per core
SB = 1024                          # s-block size
NBLK = S // SB                     # 2 s-blocks per batch
NG = B_LOC * NBLK                  # 8 s-blocks per core
NET = D // 128                     # 8 e/h tiles


def emit(tc, outs, ins, shift):
    nc = tc.nc
    ctx_out, w_out = outs
    hidden, enc, mask, W_h, b_h, W_e, b_e, v = ins

    from contextlib import ExitStack
    stack = ExitStack()
    pool = stack.enter_context(tc.tile_pool(name="sb", bufs=1))
    dpool = stack.enter_context(tc.tile_pool(name="dr", bufs=1, space="DRAM"))
    ppool = stack.enter_context(tc.tile_pool(name="ps", bufs=1, space="PSUM"))

    # ---- constant / weight setup ----
    negshift = pool.tile([128, 1], F32, name="negshift")
    nc.gpsimd.memset(negshift[:], -float(shift))

    # W_e as [p, et, h] bf16 with row e = et*128+p (cast in DMA)
    webf = pool.tile([128, NET, D], BF16, name="webf")
    nc.gpsimd.dma_start(out=webf[:], in_=W_e.rearrange("(et p) h -> p et h", p=128))
    whbf = pool.tile([128, NET, D], BF16, name="whbf")
    nc.gpsimd.dma_start(out=whbf[:], in_=W_h.rearrange("(et p) h -> p et h", p=128))
    vbf = pool.tile([128, NET], BF16, name="vbf")
    nc.gpsimd.dma_start(out=vbf[:], in_=v.rearrange("(et p) -> p et", p=128))
    hTbf = pool.tile([128, NET, B_LOC], BF16, name="hTbf")
    for b in range(B_LOC):
        nc.gpsimd.dma_start(
            out=hTbf[:, :, b], in_=hidden[b].rearrange("(k p) -> p k", p=128)
        )

    bh_sb = pool.tile([128, NET], F32, name="bh_sb")
    nc.sync.dma_start(out=bh_sb[:], in_=b_h.rearrange("(k p) -> p k", p=128))
    be_sb = pool.tile([128, NET], F32, name="be_sb")
    nc.sync.dma_start(out=be_sb[:], in_=b_e.rearrange("(k p) -> p k", p=128))
    bsum = pool.tile([128, NET], F32, name="bsum")
    nc.vector.tensor_add(bsum[:], bh_sb[:], be_sb[:])

    # ---- h_proj: hpb[p, m, b] = (hidden @ W_h)[b, m*128+p] + b_h + b_e ----
    hpb = pool.tile([128, NET, B_LOC], F32, name="hpb")
    for m in range(NET):
        psh = ppool.tile([128, B_LOC], F32, name=f"psh{m}", tag="vd", bufs=1)
        for k in range(NET):
            nc.tensor.matmul(
                psh[:],
                whbf[:, k, m * 128:(m + 1) * 128],
                hTbf[:, k, :],
                start=(k == 0), stop=(k == NET - 1),
            )
        nc.scalar.activation(hpb[:, m, :], psh[:], AF.Identity, bias=bsum[:, m:m + 1])

    # ---- pipelined main loop over s-blocks ----
    natbf = {}     # g -> list of 8 [128,1024] bf16 tiles
    btt = {}       # g -> [128, 8, 1024] bf16 block-transposed
    pTt = {}       # g -> [128, 16] bf16
    p32t = {}      # g -> [1, 1024] f32 (masked exp)
    dparts = {}    # b -> [1, 2] f32
    p32dram = {}   # b -> [1, 2048] f32 DRAM
    psctx = {}     # b -> [1, 1024] f32 PSUM

    def emit_preload(g):
        b, k = divmod(g, NBLK)
        tiles = []
        for j in range(NET):
            t = pool.tile([128, SB], BF16, name=f"nat{g}_{j}", tag="natbf", bufs=34)
            s0 = k * SB + j * 128
            nc.gpsimd.dma_start(out=t[:], in_=enc[b, s0:s0 + 128, :])
            tiles.append(t)
        natbf[g] = tiles

    def emit_transpose(g):
        bt = pool.tile([128, NET, SB], BF16, name=f"bt{g}", tag="bt", bufs=2)
        for j in range(NET):
            nc.sync.dma_start_transpose(
                out=bt[:, :, j * 128:(j + 1) * 128], in_=natbf[g][j][:]
            )
        btt[g] = bt

    def emit_compute(g):
        b, k = divmod(g, NBLK)
        bt = btt[g]
        energy = []
        for ht in range(NET):
            ps = ppool.tile([128, SB], F32, name=f"pe{g}_{ht}", tag="pe", bufs=2)
            for half in range(2):
                o = half * 512
                for et in range(NET):
                    nc.tensor.matmul(
                        ps[:, o:o + 512],
                        webf[:, et, ht * 128:(ht + 1) * 128],
                        bt[:, et, o:o + 512],
                        start=(et == 0), stop=(et == NET - 1),
                        skip_group_check=True,
                    )
            en = pool.tile([128, SB], BF16, name=f"en{g}_{ht}", tag="en", bufs=4)
            nc.scalar.activation(en[:], ps[:], AF.Tanh, bias=hpb[:, ht, b:b + 1])
            energy.append(en)

        psv = ppool.tile([1, SB], F32, name=f"psv{g}", tag="vd", bufs=1)
        for ht in range(NET):
            for half in range(2):
                o = half * 512
                nc.tensor.matmul(
                    psv[0:1, o:o + 512],
                    vbf[:, ht:ht + 1],
                    energy[ht][:, o:o + 512],
                    start=(ht == 0), stop=(ht == NET - 1),
                    skip_group_check=True,
                )

        # p = exp(logits - shift) * mask
        mi = pool.tile([1, SB], I32, name=f"mi{g}", tag="mi", bufs=2)
        nc.sync.dma_start(out=mi[:], in_=mask[b, k * SB:(k + 1) * SB])
        mf = pool.tile([1, SB], F32, name=f"mf{g}", tag="mf", bufs=2)
        nc.vector.tensor_copy(mf[:], mi[:])
        p32 = pool.tile([1, SB], F32, name=f"p32{g}", tag="p32", bufs=2)
        nc.scalar.activation(p32[:], psv[0:1, :], AF.Exp, bias=negshift[0:1, :])
        nc.vector.tensor_mul(p32[:], p32[:], mf[:])
        if k == 0:
            dparts[b] = pool.tile([1, NBLK], F32, name=f"dp{b}", tag="dp", bufs=2)
            p32dram[b] = dpool.tile([1, S], F32, name=f"p32d{b}", tag="p32d", bufs=2)
        nc.vector.reduce_sum(dparts[b][0:1, k:k + 1], p32[:], axis=mybir.AxisListType.X)
        pbf = pool.tile([1, SB], BF16, name=f"pbf{g}", tag="pbf", bufs=2)
        nc.vector.tensor_copy(pbf[:], p32[:])
        nc.sync.dma_start(out=p32dram[b][0:1, k * SB:(k + 1) * SB], in_=p32[:])

        # transpose p via tiny DRAM round trip (xbar needs >=16 rows; rows 8..15 junk)
        pd = dpool.tile([16, 128], BF16, name=f"pd{g}", tag="pd", bufs=2)
        nc.sync.dma_start(out=pd[0:8, :], in_=pbf[:])
        pT = pool.tile([128, 16], BF16, name=f"pT{g}", tag="pT", bufs=2)
        nc.sync.dma_start_transpose(out=pT[:], in_=pd[:])
        pTt[g] = pT
        p32t[g] = p32

    def emit_ctx(g):
        b, k = divmod(g, NBLK)
        if k == 0:
            psctx[b] = ppool.tile([1, D], F32, name=f"psctx{b}", tag="ctx", bufs=1)
        pc = psctx[b]
        for j in range(NET):
            for half in range(2):
                o = half * 512
                nc.tensor.matmul(
                    pc[0:1, o:o + 512],
                    pTt[g][:, j:j + 1],
                    natbf[g][j][:, o:o + 512],
                    start=(k == 0 and j == 0), stop=(k == NBLK - 1 and j == NET - 1),
                    skip_group_check=True,
                )

    def emit_batchend(b):
        dsum = pool.tile([1, 1], F32, name=f"ds{b}", tag="ds", bufs=2)
        nc.vector.reduce_sum(dsum[:], dparts[b][:], axis=mybir.AxisListType.X)
        rcp = pool.tile([1, 1], F32, name=f"rcp{b}", tag="rcp", bufs=2)
        nc.vector.reciprocal(rcp[:], dsum[:])
        ctxsb = pool.tile([1, D], F32, name=f"ctxsb{b}", tag="ctxsb", bufs=2)
        nc.scalar.activation(ctxsb[:], psctx[b][:], AF.Copy, scale=rcp[0:1, :])
        nc.sync.dma_start(out=ctx_out[b, :], in_=ctxsb[:])
        for k in range(NBLK):
            wt = pool.tile([1, SB], F32, name=f"wt{b}_{k}", tag="wt", bufs=2)
            nc.sync.dma_start(out=wt[:], in_=p32dram[b][0:1, k * SB:(k + 1) * SB])
            nc.vector.tensor_scalar_mul(wt[:], wt[:], rcp[0:1, :])
            nc.sync.dma_start(out=w_out[b, k * SB:(k + 1) * SB], in_=wt[:])

    for i in range(-2, NG + 1):
        if 0 <= i + 2 < NG:
            emit_preload(i + 2)
        if 0 <= i + 1 < NG:
            emit_transpose(i + 1)
        if 0 <= i < NG:
            emit_compute(i)
        if 0 <= i - 1 < NG:
            emit_ctx(i - 1)
            if (i - 1) % NBLK == NBLK - 1:
                emit_batchend((i - 1) // NBLK)

    stack.close()


def build_nc(shift):
    nc = bacc.Bacc("TRN2", target_bir_lowering=False, debug=False)
    ins = [
        nc.dram_tensor("hidden", [B_LOC, D], F32, kind="ExternalInput").ap(),
        nc.dram_tensor("encoder_outputs", [B_LOC, S, D], F32, kind="ExternalInput").ap(),
        nc.dram_tensor("mask", [B_LOC, S], I32, kind="ExternalInput").ap(),
        nc.dram_tensor("W_h", [D, D], F32, kind="ExternalInput").ap(),
        nc.dram_tensor("b_h", [D], F32, kind="ExternalInput").ap(),
        nc.dram_tensor("W_e", [D, D], F32, kind="ExternalInput").ap(),
        nc.dram_tensor("b_e", [D], F32, kind="ExternalInput").ap(),
        nc.dram_tensor("v", [D], F32, kind="ExternalInput").ap(),
    ]
    outs = [
        nc.dram_tensor("ctx_out", [B_LOC, D], F32, kind="ExternalOutput").ap(),
        nc.dram_tensor("w_out", [B_LOC, S], F32, kind="ExternalOutput").ap(),
    ]
    with tile.TileContext(nc) as tc:
        emit(tc, outs, ins, shift)
    nc.compile()
    return nc


def kernel(hidden, encoder_outputs, mask, W_h, b_h, W_e, b_e, v, trace=False):
    from concourse.bass_utils import run_bass_kernel_spmd

    hidden = np.ascontiguousarray(np.asarray(hidden, dtype=np.float32))
    encoder_outputs = np.asarray(encoder_outputs, dtype=np.float32)
    mask = np.asarray(mask, dtype=np.int32)
    W_h = np.ascontiguousarray(np.asarray(W_h, dtype=np.float32))
    b_h = np.ascontiguousarray(np.asarray(b_h, dtype=np.float32))
    W_e = np.ascontiguousarray(np.asarray(W_e, dtype=np.float32))
    b_e = np.ascontiguousarray(np.asarray(b_e, dtype=np.float32))
    v = np.ascontiguousarray(np.asarray(v, dtype=np.float32))

    shift = float(np.abs(v).sum())
    nc = build_nc(shift)

    in_maps = []
    for c in range(N_CORES):
        sl = slice(c * B_LOC, (c + 1) * B_LOC)
        in_maps.append({
            "hidden": np.ascontiguousarray(hidden[sl]),
            "encoder_outputs": np.ascontiguousarray(encoder_outputs[sl]),
            "mask": np.ascontiguousarray(mask[sl]),
            "W_h": W_h, "b_h": b_h, "W_e": W_e, "b_e": b_e, "v": v,
        })

    res = run_bass_kernel_spmd(nc, in_maps, list(range(N_CORES)), trace=trace)
    context = np.concatenate([res.results[c]["ctx_out"] for c in range(N_CORES)], axis=0)
    weights = np.concatenate([res.results[c]["w_out"] for c in range(N_CORES)], axis=0)
    if trace:
        return (context, weights), res
    return (context, weights)


# revision 14
# speedup vs baseline: 338.8714x; 338.8714x over previous
"""Bahdanau attention Trainium2 kernel.

Shapes (full): hidden (32,1024) f32, encoder_outputs (32,2048,1024) f32,
mask (32,2048) i32, W_h/W_e (1024,1024) f32, b_h/b_e/v (1024,) f32.
Outputs: context (32,1024) f32, attention_weights (32,2048) f32.

Sharding: data-parallel over batch B across 8 cores (4 batches/core);
projection weights replicated.

Per-core pipeline (all big compute in bf16 on the PE):
  h_projT = W_h^T @ hidden^T + b_h + b_e          (tiny, startup)
  per s-block of 1024 (2 per batch, 8 per core):
    natbf  <- gpsimd cast-DMA f32->bf16 of enc rows      [128,1024] x8
    bt     <- xbar transpose (3D out) of natbf           [128,8(et),1024(s)]
    e_projT[h,s] = sum_et W_e[et]^T @ bt[et]  (PSUM f32)
    energy = tanh(e_projT + bias)  (ACT, per-partition bias)
    logits[s] = sum_ht v[ht]^T @ energy[ht]   (M=1 matmuls)
    p = exp(logits - |v|_1) * mask   (ACT exp + DVE mask-mul; no row max needed
                                      since |logit| <= |v|_1)
    pT <- tiny DRAM round-trip xbar ( [16,128] -> [128,16] )
    ctx += sum_j pT[:,j]^T @ natbf[j]          (M=1 matmuls, deferred 1 block)
  ctx_out = ctx * (1/sum p);  w_out = p * (1/sum p)
"""

import sys

for _p in ("/opt/trn_rl_repo", "/root/.axon_site/_ro/trn_rl_repo"):
    if _p not in sys.path:
        sys.path.insert(0, _p)

import numpy as np

import concourse.bacc as bacc
import concourse.mybir as mybir
import concourse.tile as tile

F32 = mybir.dt.float32
BF16 = mybir.dt.bfloat16
I32 = mybir.dt.int32
AF = mybir.ActivationFunctionType

N_CORES = 8
B_FULL, S, D = 32, 2048, 1024
B_LOC = B_FULL // N_CORES          # 4 batches per core
SB = 1024                          # s-block size
NBLK = S // SB                     # 2 s-blocks per batch
NG = B_LOC * NBLK                  # 8 s-blocks per core
NET = D // 128                     # 8 e/h tiles


def emit(tc, outs, ins, shift, repeat=1):
    nc = tc.nc
    ctx_out, w_out = outs
    hidden, enc, mask, W_h, b_h, W_e, b_e, v = ins

    from contextlib import ExitStack
    stack = ExitStack()
    pool = stack.enter_context(tc.tile_pool(name="sb", bufs=1))
    dpool = stack.enter_context(tc.tile_pool(name="dr", bufs=1, space="DRAM"))
    ppool = stack.enter_context(tc.tile_pool(name="ps", bufs=1, space="PSUM"))

    # ---- constant / weight setup ----
    negshift = pool.tile([128, 1], F32, name="negshift")
    nc.gpsimd.memset(negshift[:], -float(shift))

    # small h_proj-critical loads first, then weights (W_e split per chunk so
    # the first e_proj matmuls only wait on chunk 0)
    hTbf = pool.tile([128, NET, B_LOC], BF16, name="hTbf")
    for b in range(B_LOC):
        nc.gpsimd.dma_start(
            out=hTbf[:, :, b], in_=hidden[b].rearrange("(k p) -> p k", p=128)
        )
    vbf = pool.tile([128, NET], BF16, name="vbf")
    nc.gpsimd.dma_start(out=vbf[:], in_=v.rearrange("(et p) -> p et", p=128))
    # Weights via HWDGE f32 loads + DVE cast: keeps the SWDGE rail free for
    # the enc cast-DMA preloads during warmup. Layout [p, et, h], row e=et*128+p.
    whbf = pool.tile([128, NET, D], BF16, name="whbf")
    webf = pool.tile([128, NET, D], BF16, name="webf")
    for (wsrc, wdst) in ((W_h, whbf), (W_e, webf)):
        for et in range(NET):
            wf = pool.tile([128, D], F32, name=f"wf{et}", tag="wf32", bufs=2)
            nc.sync.dma_start(out=wf[:], in_=wsrc[et * 128:(et + 1) * 128, :])
            nc.vector.tensor_copy(wdst[:, et, :], wf[:])

    bh_sb = pool.tile([128, NET], F32, name="bh_sb")
    nc.sync.dma_start(out=bh_sb[:], in_=b_h.rearrange("(k p) -> p k", p=128))
    be_sb = pool.tile([128, NET], F32, name="be_sb")
    nc.sync.dma_start(out=be_sb[:], in_=b_e.rearrange("(k p) -> p k", p=128))
    bsum = pool.tile([128, NET], F32, name="bsum")
    nc.vector.tensor_add(bsum[:], bh_sb[:], be_sb[:])

    # ---- h_proj: hpb[p, m, b] = (hidden @ W_h)[b, m*128+p] + b_h + b_e ----
    hpb = pool.tile([128, NET, B_LOC], F32, name="hpb")
    for m in range(NET):
        psh = ppool.tile([128, B_LOC], F32, name=f"psh{m}", tag="vd", bufs=1)
        for k in range(NET):
            nc.tensor.matmul(
                psh[:],
                whbf[:, k, m * 128:(m + 1) * 128],
                hTbf[:, k, :],
                start=(k == 0), stop=(k == NET - 1),
            )
        nc.scalar.activation(hpb[:, m, :], psh[:], AF.Identity, bias=bsum[:, m:m + 1])

    # ---- pipelined main loop over s-blocks ----
    natbf = {}     # g -> list of 8 [128,1024] bf16 tiles
    btt = {}       # g -> [128, 8, 1024] bf16 block-transposed
    pTt = {}       # g -> [128, 16] bf16
    dparts = {}    # b -> [1, 2] f32
    p32dram = {}   # b -> [1, 2048] f32 DRAM
    psctx = {}     # b -> [1, 1024] f32 PSUM

    def emit_preload(g):
        b, k = divmod(g, NBLK)
        tiles = []
        for j in range(NET):
            t = pool.tile([128, SB], BF16, name=f"nat{g}_{j}", tag="natbf", bufs=32)
            s0 = k * SB + j * 128
            nc.gpsimd.dma_start(out=t[:], in_=enc[b, s0:s0 + 128, :])
            tiles.append(t)
        natbf[g] = tiles

    def emit_transpose(g):
        bt = pool.tile([128, NET, SB], BF16, name=f"bt{g}", tag="bt", bufs=2)
        for j in range(NET):
            nc.sync.dma_start_transpose(
                out=bt[:, :, j * 128:(j + 1) * 128], in_=natbf[g][j][:]
            )
        btt[g] = bt

    def emit_compute(g):
        b, k = divmod(g, NBLK)
        bt = btt[g]
        energy = []
        psv = ppool.tile([1, SB], F32, name=f"psv{g}", tag="vd", bufs=1)

        def emit_vdot(ht):
            for half in range(2):
                o = half * 512
                nc.tensor.matmul(
                    psv[0:1, o:o + 512],
                    vbf[:, ht:ht + 1],
                    energy[ht][:, o:o + 512],
                    start=(ht == 0), stop=(ht == NET - 1),
                    skip_group_check=True,
                )

        for ht in range(NET):
            ps = ppool.tile([128, SB], F32, name=f"pe{g}_{ht}", tag="pe", bufs=2)
            for half in range(2):
                o = half * 512
                for et in range(NET):
                    nc.tensor.matmul(
                        ps[:, o:o + 512],
                        webf[:, et, ht * 128:(ht + 1) * 128],
                        bt[:, et, o:o + 512],
                        start=(et == 0), stop=(et == NET - 1),
                        skip_group_check=True,
                    )
            en = pool.tile([128, SB], BF16, name=f"en{g}_{ht}", tag="en", bufs=4)
            nc.scalar.activation(en[:], ps[:], AF.Tanh, bias=hpb[:, ht, b:b + 1])
            energy.append(en)
            # v-dot lags one h-tile behind e_proj so tanh(ht) overlaps
            # e_proj(ht+1) on the PE and energy slots free continuously
            if ht >= 1:
                emit_vdot(ht - 1)
        emit_vdot(NET - 1)

        # p = exp(logits - shift) * mask
        mi = pool.tile([1, SB], I32, name=f"mi{g}", tag="mi", bufs=2)
        nc.sync.dma_start(out=mi[:], in_=mask[b, k * SB:(k + 1) * SB])
        mf = pool.tile([1, SB], F32, name=f"mf{g}", tag="mf", bufs=2)
        nc.vector.tensor_copy(mf[:], mi[:])
        p32 = pool.tile([1, SB], F32, name=f"p32{g}", tag="p32", bufs=2)
        nc.scalar.activation(p32[:], psv[0:1, :], AF.Exp, bias=negshift[0:1, :])
        nc.vector.tensor_mul(p32[:], p32[:], mf[:])
        if k == 0:
            dparts[b] = pool.tile([1, NBLK], F32, name=f"dp{b}", tag="dp", bufs=2)
            p32dram[b] = dpool.tile([1, S], F32, name=f"p32d{b}", tag="p32d", bufs=2)
        nc.vector.reduce_sum(dparts[b][0:1, k:k + 1], p32[:], axis=mybir.AxisListType.X)
        pbf = pool.tile([1, SB], BF16, name=f"pbf{g}", tag="pbf", bufs=2)
        nc.vector.tensor_copy(pbf[:], p32[:])
        nc.sync.dma_start(out=p32dram[b][0:1, k * SB:(k + 1) * SB], in_=p32[:])

        # transpose p via tiny DRAM round trip (xbar needs >=16 rows; rows 8..15 junk)
        pd = dpool.tile([16, 128], BF16, name=f"pd{g}", tag="pd", bufs=2)
        nc.sync.dma_start(out=pd[0:8, :], in_=pbf[:])
        pT = pool.tile([128, 16], BF16, name=f"pT{g}", tag="pT", bufs=2)
        nc.sync.dma_start_transpose(out=pT[:], in_=pd[:])
        pTt[g] = pT

    def emit_ctx(g):
        b, k = divmod(g, NBLK)
        if k == 0:
            psctx[b] = ppool.tile([1, D], F32, name=f"psctx{b}", tag="ctx", bufs=1)
        pc = psctx[b]
        for j in range(NET):
            for half in range(2):
                o = half * 512
                nc.tensor.matmul(
                    pc[0:1, o:o + 512],
                    pTt[g][:, j:j + 1],
                    natbf[g][j][:, o:o + 512],
                    start=(k == 0 and j == 0), stop=(k == NBLK - 1 and j == NET - 1),
                    skip_group_check=True,
                )

    def emit_batchend(b):
        dsum = pool.tile([1, 1], F32, name=f"ds{b}", tag="ds", bufs=2)
        nc.vector.reduce_sum(dsum[:], dparts[b][:], axis=mybir.AxisListType.X)
        rcp = pool.tile([1, 1], F32, name=f"rcp{b}", tag="rcp", bufs=2)
        nc.vector.reciprocal(rcp[:], dsum[:])
        ctxsb = pool.tile([1, D], F32, name=f"ctxsb{b}", tag="ctxsb", bufs=2)
        nc.scalar.activation(ctxsb[:], psctx[b][:], AF.Copy, scale=rcp[0:1, :])
        nc.sync.dma_start(out=ctx_out[b, :], in_=ctxsb[:])
        for k in range(NBLK):
            wt = pool.tile([1, SB], F32, name=f"wt{b}_{k}", tag="wt", bufs=2)
            nc.sync.dma_start(out=wt[:], in_=p32dram[b][0:1, k * SB:(k + 1) * SB])
            nc.vector.tensor_scalar_mul(wt[:], wt[:], rcp[0:1, :])
            nc.sync.dma_start(out=w_out[b, k * SB:(k + 1) * SB], in_=wt[:])

    for _rep in range(repeat):
        natbf.clear(); btt.clear(); pTt.clear()
        dparts.clear(); p32dram.clear(); psctx.clear()
        for i in range(-2, NG + 1):
            if 0 <= i + 2 < NG:
                emit_preload(i + 2)
            if 0 <= i + 1 < NG:
                emit_transpose(i + 1)
            if 0 <= i < NG:
                emit_compute(i)
            if 0 <= i - 1 < NG:
                emit_ctx(i - 1)
                if (i - 1) % NBLK == NBLK - 1:
                    emit_batchend((i - 1) // NBLK)

    stack.close()


def build_nc(shift, repeat=1):
    nc = bacc.Bacc("TRN2", target_bir_lowering=False, debug=False)
    ins = [
        nc.dram_tensor("hidden", [B_LOC, D], F32, kind="ExternalInput").ap(),
        nc.dram_tensor("encoder_outputs", [B_LOC, S, D], F32, kind="ExternalInput").ap(),
        nc.dram_tensor("mask", [B_LOC, S], I32, kind="ExternalInput").ap(),
        nc.dram_tensor("W_h", [D, D], F32, kind="ExternalInput").ap(),
        nc.dram_tensor("b_h", [D], F32, kind="ExternalInput").ap(),
        nc.dram_tensor("W_e", [D, D], F32, kind="ExternalInput").ap(),
        nc.dram_tensor("b_e", [D], F32, kind="ExternalInput").ap(),
        nc.dram_tensor("v", [D], F32, kind="ExternalInput").ap(),
    ]
    outs = [
        nc.dram_tensor("ctx_out", [B_LOC, D], F32, kind="ExternalOutput").ap(),
        nc.dram_tensor("w_out", [B_LOC, S], F32, kind="ExternalOutput").ap(),
    ]
    with tile.TileContext(nc) as tc:
        emit(tc, outs, ins, shift, repeat=repeat)
    nc.compile()
    return nc


def kernel(hidden, encoder_outputs, mask, W_h, b_h, W_e, b_e, v, trace=False):
    from concourse.bass_utils import run_bass_kernel_spmd

    hidden = np.ascontiguousarray(np.asarray(hidden, dtype=np.float32))
    encoder_outputs = np.asarray(encoder_outputs, dtype=np.float32)
    mask = np.asarray(mask, dtype=np.int32)
    W_h = np.ascontiguousarray(np.asarray(W_h, dtype=np.float32))
    b_h = np.ascontiguousarray(np.asarray(b_h, dtype=np.float32))
    W_e = np.ascontiguousarray(np.asarray(W_e, dtype=np.float32))
    b_e = np.ascontiguousarray(np.asarray(b_e, dtype=np.float32))
    v = np.ascontiguousarray(np.asarray(v, dtype=np.float32))

    shift = float(np.abs(v).sum())
    nc = build_nc(shift)

    in_maps = []
    for c in range(N_CORES):
        sl = slice(c * B_LOC, (c + 1) * B_LOC)
        in_maps.append({
            "hidden": np.ascontiguousarray(hidden[sl]),
            "encoder_outputs": np.ascontiguousarray(encoder_outputs[sl]),
            "mask": np.ascontiguousarray(mask[sl]),
            "W_h": W_h, "b_h": b_h, "W_e": W_e, "b_e": b_e, "v": v,
        })

    res = run_bass_kernel_spmd(nc, in_maps, list(range(N_CORES)), trace=trace)
    context = np.concatenate([res.results[c]["ctx_out"] for c in range(N_CORES)], axis=0)
    weights = np.concatenate([res.results[c]["w_out"] for c in range(N_CORES)], axis=0)
    if trace:
        return (context, weights), res
    return (context, weights)
